# revision 35
# baseline (speedup 1.0000x reference)
"""Trainium2 Bass kernel for nn_GCLSTMModel_48868137894020.

The reference runs ONE GCLSTM cell step per layer with H0 = C0 = 0, so:
  - conv(k, H) = H @ cW[k,0] + (Lhat @ H) @ cW[k,1] + cb[k]  ==  cb[k]
    (both matmul terms multiply a zero matrix) -> cheb weights and the
    graph Laplacian are algebraically dead,
  - the forget gate output is multiplied by C = 0 -> dead,
  - peep[0] * C and peep[1] * C are 0 -> dead.
What remains per layer (d = 140 then 280), with X the layer input:
  I = sigmoid(X @ W[0] + cb[0] + b[0])
  T = tanh   (X @ W[2] + cb[2] + b[2])
  C = I * T
  O = sigmoid(X @ W[3] + cb[3] + b[3] + peep[2] * C)
  X' = relu(O * tanh(C))        (== max(tanh(C), 0) * O  since O > 0)
then out = relu(X'' @ fc_w + fc_b).

Kernel layout strategy (single NeuronCore program, replicated on 8 cores):
  - ALL matmul operands are bf16 (halves DMA bytes, doubles PE rate);
    gate elementwise math and PSUM accumulation stay fp32. Worst-case
    relative error ~0.5% vs the 2e-2 gate.
  - L1 computed TRANSPOSED (features on partitions): x1T = W1[k].T @ adj
    (adj is symmetric by construction; 0/1 entries are exact in bf16).
    The contraction is extended from 35 to 38 rows: rows 35/36 hold
    cb1/b1 against all-ones rhs columns (bias add for free), row 37
    holds peep1[2] against a one-hot rhs column, so psum col 36 of the
    I-gate IS the per-partition peephole vector (copied to SBUF for the
    STT scalar operand).
  - L2 computed NORMAL (nodes on partitions): lhsT = x1T chunks directly
    (no transpose needed); cb2/b2 are folded in via constant-1 rows of
    x1T whose weight rows ride in the w2x tensor.
  - FC needs x2^T: 3 PE transposes (bf16), then 3 accumulating matmuls;
    fc_b is folded in via another constant-1 row.

DMA engineering (measured: all HWDGE queues stripe their per-partition
lines over 16 shared ~25 GB/s DMA engines which round-robin between
queues, so concurrent DMAs share bandwidth; ~0.7us issue->wire latency):
  - six right-sized DMAs issued in URGENCY order: g1 (L1 block) rides
    the scalar ring alone; w2x/w2m/pe2b/fcwx ride the sync ring in
    need order; fcw's issue is pinned BEHIND the L1 activations in the
    scalar queue so its 128 tiny lines don't contend with w2m.
  - peep2 row is broadcast to 35 partitions on the HOST (pure data
    replication); all other tensors are exact.
  - sigmoid AND tanh both live in the 'sigmoid_and_others' activation
    table; a Bacc subclass hands insert_act_table_loads a table list
    where every other set is EMPTY (original indices preserved) so ONE
    ACT_TABLE_LOAD is emitted instead of two serialized 1.3us loads.

Sharding: the problem is tiny (N=35) and an on-device collective has
~15us constant overhead; all 8 cores run the identical program on
replicated inputs (no collectives), output taken from core 0.
"""

import sys

for _p in ("/opt/trn_rl_repo", "/opt/pypackages"):
    if _p not in sys.path:
        sys.path.append(_p)

from contextlib import ExitStack

import ml_dtypes
import numpy as np

import bass_rust as _bass_rust
import concourse.bacc as bacc
import concourse.bass as bass
import concourse.mybir as mybir
import concourse.tile as tile
from concourse.hw_specs import get_activation_tables

F32 = mybir.dt.float32
BF16 = mybir.dt.bfloat16
AF = mybir.ActivationFunctionType
OP = mybir.AluOpType
GATES = (0, 2, 3)  # I, T (cell), O — forget gate (1) is dead
N = 35
D1 = 140
D2 = 280
N_CORES = 8
BF = ml_dtypes.bfloat16

# g1 [40, 494] bf16: cols 0:38 rhs block (adj + bias-ones rows + peep
# one-hot col), 38:73 identity, 73:493 W1 gates (rows 35/36/37 = cb1 /
# b1 / peep1[2]), col 493 pad. Rows 38:40 are zero padding: 40 lines
# chunk as 4-per-engine across 10 SDMA engines (38 = 2x19 would go
# 19-per-engine across 2).
G1_ROWS = 40
G1_COLS = 494


class _BaccOneActTable(bacc.Bacc):
    """Bacc that loads ONE activation table: 'sigmoid_and_others' holds
    every function this kernel uses (sigmoid, tanh). The stock pass
    assigns tanh to the 'exp_and_others' set, emitting two serialized
    ACT_TABLE_LOADs; handing it a table list where every other set is
    empty (list order — and therefore act_func_set_id — unchanged)
    forces a single load."""

    def insert_act_table_loads(self):
        has_activation = any(
            isinstance(i, mybir.InstActivation)
            for b in self.main_func.blocks
            for i in b.instructions
        )
        if not has_activation:
            return
        tables = [
            (name, funcs if name == "sigmoid_and_others" else set())
            for name, funcs in get_activation_tables(self.m.arch).items()
        ]
        _bass_rust.insert_act_table_loads(self, tables)


def build_nc() -> bass.Bass:
    nc = _BaccOneActTable()

    g1 = nc.dram_tensor("g1", [G1_ROWS, G1_COLS], BF16, kind="ExternalInput")
    w2x = nc.dram_tensor("w2x", [14, 3 * D2], BF16, kind="ExternalInput")
    w2m = nc.dram_tensor("w2m", [128, 3 * D2], BF16, kind="ExternalInput")
    pe2b = nc.dram_tensor("pe2b", [N, D2], F32, kind="ExternalInput")
    # fcA [128, 108]: cols 0:72 = fc_w row-chunks 0/1 (35 cols + pad each),
    # cols 72:108 rows 0:25 = [fc_w[256:280]; fc_b] (the x2T2 lhsT chunk)
    fcA = nc.dram_tensor("fcA", [128, 108], BF16, kind="ExternalInput")
    out = nc.dram_tensor("out", [N, N], F32, kind="ExternalOutput")

    with ExitStack() as ctx:
        tc = ctx.enter_context(tile.TileContext(nc))
        sb = ctx.enter_context(tc.tile_pool(name="sb", bufs=1))
        psA = ctx.enter_context(tc.tile_pool(name="psA", bufs=5, space="PSUM"))
        psB = ctx.enter_context(tc.tile_pool(name="psB", bufs=3, space="PSUM"))

        # ---- input DMAs in urgency order (g1 alone on the scalar ring) ----
        # HWDGE splits a DMA's P lines into chunks of (smallest divisor of P
        # that is >= P/16), one chunk per ~23GB/s SDMA engine — prime or
        # large-divisor line counts serialize onto 1-2 engines. Pick counts
        # with small divisors and split the big tensors across both rings:
        # g1 = 20+18 rows (10+9 engines), w2m = 64+64 (16 engines each).
        # descgen occupies the ISSUING engine for 0.6-1.6us per DMA (the two
        # rings' descgens also serialize against each other in the shared
        # HWDGE unit) — so: g1 alone on the scalar ring as ONE wide-striped
        # DMA, everything else on the otherwise-idle sync ring in need order.
        g1_sb = sb.tile([G1_ROWS, G1_COLS], BF16, tag="g1")
        d_g1 = nc.sync.dma_start(out=g1_sb, in_=g1[:, :])
        w2x_sb = sb.tile([14, 3 * D2], BF16, tag="w2x")
        d_w2x = nc.sync.dma_start(out=w2x_sb, in_=w2x[:, :])
        tile.add_dep_helper(d_w2x.ins, d_g1.ins, sync=False, reason="dma order")
        w2m_sb = sb.tile([128, 3 * D2], BF16, tag="w2m")
        d_w2m = nc.sync.dma_start(out=w2m_sb, in_=w2m[:, :])
        tile.add_dep_helper(d_w2m.ins, d_w2x.ins, sync=False, reason="dma order")
        pe2_sb = sb.tile([N, D2], F32, tag="pe2")
        d_pe2 = nc.sync.dma_start(out=pe2_sb, in_=pe2b[:, :])
        tile.add_dep_helper(d_pe2.ins, d_w2m.ins, sync=False, reason="dma order")
        fcA_sb = sb.tile([128, 108], BF16, tag="fcA")
        d_fcA = nc.sync.dma_start(out=fcA_sb, in_=fcA[:, :])
        tile.add_dep_helper(d_fcA.ins, d_pe2.ins, sync=False, reason="dma order")
        fcw_sb = fcA_sb
        fcwx_sb = fcA_sb[0:25, 72:108]

        rhs1_v = g1_sb[:, 0:38]
        ident_v = g1_sb[0:35, 38:73]

        # warm-up sigmoid with no DMA deps: forces the activation-table load
        # to run during the DMA wait instead of blocking the first real gate
        warm_in = sb.tile([1, 2], F32, tag="warm_in")
        nc.vector.memset(warm_in[:, :], 0.0)
        warm = sb.tile([1, 2], F32, tag="warm")
        nc.scalar.activation(warm, warm_in, AF.Sigmoid)

        x1T_c0 = sb.tile([128, N], BF16, tag="x1T0")
        x1T_c1 = sb.tile([14, N], BF16, tag="x1T1")  # rows 12:14 stay 1.0
        nc.vector.memset(x1T_c1[:, :], 1.0)

        # ---- layer 1, transposed: x1T[f, n] ----
        # small chunk (features 128:140) FIRST so x1T_c1 is ready early and
        # L2's accumulation groups can start on it while chunk0 gates finish
        chunks1 = ((1, 128, 140), (0, 0, 128))
        h1 = {}  # per-chunk instruction handles for queue-order pinning
        for ci, a, b in chunks1:
            cs = b - a
            if ci == 1:
                # small chunk: one packed psum tile (its matmuls finish early,
                # bank serialization is harmless)
                ps1 = psA.tile([cs, 3, 38], F32, tag="psA", name=f"ps1_{ci}")
                pviews = [ps1[:, k, :] for k in range(3)]
            else:
                # big chunk: separate banks so the I-gate activation is not
                # serialized behind the T/O matmuls writing the same bank
                pviews = [
                    psA.tile([cs, 38], F32, tag="psA", name=f"ps1_{ci}_{k}")
                    for k in range(3)
                ]
            for k in range(3):
                nc.tensor.matmul(
                    pviews[k],
                    lhsT=g1_sb[:, 73 + k * D1 + a : 73 + k * D1 + b],
                    rhs=rhs1_v,
                    start=True,
                    stop=True,
                )
            # psum col 36 of the I-gate = peep1[2] per partition (one-hot
            # rhs col against lhsT row 37); stash it for the STT scalar
            pe1c = sb.tile([cs, 1], F32, tag=f"pe1c{ci}")
            nc.vector.tensor_copy(pe1c, pviews[0][:, 36:37])
            pviews = [pv[:, 0:N] for pv in pviews]
            gi = sb.tile([cs, N], F32, tag=f"gi{ci}")
            gt = sb.tile([cs, N], F32, tag=f"gt{ci}")
            i_gi = nc.scalar.activation(gi, pviews[0], AF.Sigmoid)
            i_gt = nc.scalar.activation(gt, pviews[1], AF.Tanh)
            gc = sb.tile([cs, N], F32, tag=f"gc{ci}")
            nc.vector.tensor_mul(gc, gi, gt)
            pre_o = sb.tile([cs, N], F32, tag=f"po{ci}")
            i_po = nc.vector.scalar_tensor_tensor(
                pre_o, in0=gc, scalar=pe1c[:, 0:1],
                in1=pviews[2], op0=OP.mult, op1=OP.add,
            )
            go = sb.tile([cs, N], F32, tag=f"go{ci}")
            i_go = nc.scalar.activation(go, pre_o, AF.Sigmoid)
            tc_ = sb.tile([cs, N], F32, tag=f"tc{ci}")
            i_tc = nc.scalar.activation(tc_, gc, AF.Tanh)
            dst = x1T_c1[0:12, :] if ci == 1 else x1T_c0[:, :]
            # relu(O * tanh(C)) == max(tanh(C), 0) * O  since O = sigmoid(..) > 0
            i_stt = nc.vector.scalar_tensor_tensor(
                dst, in0=tc_, scalar=0.0, in1=go, op0=OP.max, op1=OP.mult
            )
            h1[ci] = dict(gi=i_gi, gt=i_gt, po=i_po, go=i_go, tc=i_tc, stt=i_stt)
        # pin the ACT queue to [gi1, gt1, gi0, gt0, go1, tc1, go0, tc0] and
        # keep x1T_c1's STT ahead of c0's pre_o on DVE — otherwise the
        # scheduler interleaves the chunks so x1T_c1 lands LAST and the L2
        # c1-part matmuls can't overlap the c0 gate tail (~0.5us).
        tile.add_dep_helper(
            h1[0]["gi"].ins, h1[1]["gt"].ins, sync=False, reason="L1 act order"
        )
        tile.add_dep_helper(
            h1[1]["go"].ins, h1[0]["gt"].ins, sync=False, reason="L1 act order"
        )
        tile.add_dep_helper(
            h1[0]["go"].ins, h1[1]["tc"].ins, sync=False, reason="L1 act order"
        )
        tile.add_dep_helper(
            h1[0]["po"].ins, h1[1]["stt"].ins, sync=False, reason="L1 dve order"
        )

        # ---- layer 2, normal: x2[n, f] ----
        ps2 = [psB.tile([N, D2], F32, tag="psB", name=f"ps2_{k}") for k in range(3)]
        for k in range(3):
            # x1T_c1 part first: it's ready before the big chunk0 gates finish
            nc.tensor.matmul(
                ps2[k],
                lhsT=x1T_c1[:, :],
                rhs=w2x_sb[:, k * D2 : (k + 1) * D2],
                start=True,
                stop=False,
            )
            nc.tensor.matmul(
                ps2[k],
                lhsT=x1T_c0[:, :],
                rhs=w2m_sb[:, k * D2 : (k + 1) * D2],
                start=False,
                stop=True,
            )
        # gates in two column halves: pipelines ACT against DVE and lets the
        # first x2 transpose start as soon as cols 0:140 are done
        i2 = sb.tile([N, D2], F32, tag="i2")
        t2 = sb.tile([N, D2], F32, tag="t2")
        c2 = sb.tile([N, D2], F32, tag="c2")
        pc2 = sb.tile([N, D2], F32, tag="pc2")
        pre_o2 = sb.tile([N, D2], F32, tag="preo2")
        o2 = sb.tile([N, D2], F32, tag="o2")
        tc2 = sb.tile([N, D2], F32, tag="tc2")
        x2 = sb.tile([N, D2], BF16, tag="x2")
        x2_stt_last = None
        o2_first = None
        tc2_last = None
        for ha, hb in ((0, 140), (140, D2)):
            h = slice(ha, hb)
            nc.scalar.activation(i2[:, h], ps2[0][:, h], AF.Sigmoid)
            nc.scalar.activation(t2[:, h], ps2[1][:, h], AF.Tanh)
            nc.vector.tensor_mul(c2[:, h], i2[:, h], t2[:, h])
            nc.vector.tensor_mul(pc2[:, h], c2[:, h], pe2_sb[:, h])
            nc.vector.tensor_add(pre_o2[:, h], pc2[:, h], ps2[2][:, h])
            i_o2 = nc.scalar.activation(o2[:, h], pre_o2[:, h], AF.Sigmoid)
            if o2_first is None:
                o2_first = i_o2
            tc2_last = nc.scalar.activation(tc2[:, h], c2[:, h], AF.Tanh)
            x2_stt_last = nc.vector.scalar_tensor_tensor(
                x2[:, h], in0=tc2[:, h], scalar=0.0, in1=o2[:, h],
                op0=OP.max, op1=OP.mult,
            )
        # o2h1 ahead of tc2h2 in the ACT queue: preo2h1 is ready earlier, and
        # an early x2h1 unblocks the serial FC accumulation chain sooner
        tile.add_dep_helper(
            tc2_last.ins, o2_first.ins, sync=False, reason="o2h1 before tc2h2"
        )

        # ---- transpose x2 (PE, bf16 passthrough), then FC ----
        x2T0 = sb.tile([128, N], BF16, tag="x2T0")
        x2T1 = sb.tile([128, N], BF16, tag="x2T1")
        x2T2 = sb.tile([25, N], BF16, tag="x2T2")  # row 24 stays 1.0 (fc_b row)
        nc.vector.memset(x2T2[:, :], 1.0)
        # all transposes first (keeps PE busy, avoids conservative merged
        # waits on the cast ticks), then all psum->sbuf copies
        psTs = []
        t_last = None
        for j, (a, b) in enumerate(((0, 128), (128, 256), (256, 280))):
            psT = psA.tile([b - a, N], BF16, tag="psA", name=f"psT{j}")
            t_last = nc.tensor.transpose(psT, x2[:, a:b], ident_v)
            psTs.append(psT)
        for j, dst in enumerate((x2T0[:, :], x2T1[:, :], x2T2[0:24, :])):
            cp = nc.vector.tensor_copy(dst, psTs[j])
            # keep the copies BEHIND the x2 gate writes in the Vector stream —
            # otherwise the scheduler can order a copy (blocked on a PE
            # transpose) ahead of x2's second half, head-of-line blocking it
            tile.add_dep_helper(
                cp.ins, x2_stt_last.ins, sync=False, reason="copies after x2"
            )
        psfc = psB.tile([N, 36], F32, tag="psB", name="psfc")
        fc1 = nc.tensor.matmul(
            psfc, lhsT=x2T0[:, :], rhs=fcw_sb[:, 0:36], start=True, stop=False
        )
        # keep all transposes ahead of the FC matmuls in the PE stream — the
        # scheduler's cost model mispredicts the gate-half completion order
        # and otherwise interleaves FC1 before T2/T3, idling PE ~0.5us
        tile.add_dep_helper(
            fc1.ins, t_last.ins, sync=False, reason="transposes before FC"
        )
        nc.tensor.matmul(
            psfc, lhsT=x2T1[:, :], rhs=fcw_sb[:, 36:72], start=False, stop=False
        )
        fc3 = nc.tensor.matmul(
            psfc, lhsT=x2T2[:, :], rhs=fcwx_sb, start=False, stop=True
        )
        out_sb = sb.tile([N, N], F32, tag="out_sb")
        # relu on DVE (tensor_scalar_max with 0.0): ~110ns cheaper than the
        # ACT activation and DVE has the faster PSUM access path
        i_relu = nc.vector.tensor_scalar_max(out_sb, psfc[:, 0:N], 0.0)
        # single output DMA on the sync ring: its HWDGE is warm (first
        # descgen on an unused ring costs ~1.6us vs ~0.78 warm), and the
        # two rings' descgens serialize anyway
        d_out = nc.sync.dma_start(out=out[:, :], in_=out_sb)
        # the DMA instruction's ~0.75us descriptor generation reads no data;
        # retarget its wait from the relu to fc3's semaphore so descgen and
        # doorbell overlap the relu. The SDMA engines' first actual read of
        # out_sb trails fc3 by descgen+doorbell (>1.3us) while the relu
        # completes within ~0.3us of the same semaphore — >1us of margin.
        d_out.ins.remove_dependency(i_relu.ins.name)
        tile.add_dep_helper(
            d_out.ins, fc3.ins, sync=True, reason="descgen overlaps relu"
        )

    nc.compile()
    return nc


def pack_inputs(
    adj_matrix, W1, cheb1_b, peep1, b1, W2, cheb2_b, peep2, b2, fc_w, fc_b
) -> dict:
    """Host-side packing (gather/concat/replication + bf16 rounding)."""
    f = np.float32
    g1_h = np.zeros((G1_ROWS, G1_COLS), dtype=f)
    g1_h[0:35, 0:35] = adj_matrix
    g1_h[35, 0:36] = 1.0  # cb1 row activates on all node columns
    g1_h[36, 0:36] = 1.0  # b1 row
    g1_h[37, 36] = 1.0  # one-hot col: psum col 36 = peep1[2] per feature
    g1_h[0:35, 38:73] = np.eye(N, dtype=f)
    for i, g in enumerate(GATES):
        c = 73 + i * D1
        g1_h[0:35, c : c + D1] = W1[g]
        g1_h[35, c : c + D1] = cheb1_b[g]
        g1_h[36, c : c + D1] = b1[g]
        g1_h[37, c : c + D1] = peep1[2]

    w2x_h = np.zeros((14, 3 * D2), dtype=f)
    w2m_h = np.zeros((128, 3 * D2), dtype=f)
    for i, g in enumerate(GATES):
        w2x_h[0:12, i * D2 : (i + 1) * D2] = W2[g][128:140]
        w2x_h[12, i * D2 : (i + 1) * D2] = cheb2_b[g]
        w2x_h[13, i * D2 : (i + 1) * D2] = b2[g]
        w2m_h[:, i * D2 : (i + 1) * D2] = W2[g][0:128]

    fcA_h = np.zeros((128, 108), dtype=f)
    fcA_h[:, 0:35] = fc_w[0:128]
    fcA_h[:, 36:71] = fc_w[128:256]
    fcA_h[0:24, 72:107] = fc_w[256:280]
    fcA_h[24, 72:107] = fc_b

    return {
        "g1": g1_h.astype(BF),
        "w2x": w2x_h.astype(BF),
        "w2m": w2m_h.astype(BF),
        "pe2b": np.ascontiguousarray(
            np.broadcast_to(peep2[2], (N, D2)), dtype=f
        ),
        "fcA": fcA_h.astype(BF),
    }


_NC_CACHE: list = []


def kernel(
    adj_matrix,
    W1,
    cheb1_W,
    cheb1_b,
    peep1,
    b1,
    W2,
    cheb2_W,
    cheb2_b,
    peep2,
    b2,
    fc_w,
    fc_b,
) -> np.ndarray:
    from concourse.bass_utils import run_bass_kernel_spmd

    in_map = pack_inputs(
        adj_matrix, W1, cheb1_b, peep1, b1, W2, cheb2_b, peep2, b2, fc_w, fc_b
    )

    if not _NC_CACHE:
        _NC_CACHE.append(build_nc())
    nc = _NC_CACHE[0]

    in_maps = [dict(in_map) for _ in range(N_CORES)]
    res = run_bass_kernel_spmd(nc, in_maps, core_ids=list(range(N_CORES)))
    return np.asarray(res.results[0]["out"], dtype=np.float32)
